# revision 29
# baseline (speedup 1.0000x reference)
"""OBB label assigner on 8 Trainium2 NeuronCores (Bass/Tile SPMD kernel).

Self-contained: builds, compiles and runs the kernel; host code only
shards/replicates input layouts and concatenates outputs.

Layout per core ("layout B"): SBUF tiles [128, 682] f32 where
partition p = grp*32 + g (grp in 0..3 selects a 682-point block of the
core's 2728-point shard; g in 0..31 is the GT index). Per-GT constants are
per-partition scalars; per-point fields are replicated across the 32 g
partitions of each grp block (host prepares the replicated layout).

Rotated-box IoU: sort-free Green's-theorem formulation. Each box's 4 edges
are clipped against the other box's rect (in that box's frame) with a
scaled Liang-Barsky parameterization (no divisions), and the line-integral
cross terms are accumulated with a translation correction for the
second frame. Validated against the reference in proto2.py.
"""
import numpy as np

N = 21824
NCORES = 8
SH = N // NCORES          # 2728 points per core
FR = SH // 4              # 682 free width
G = 32
NCLS = 15
BG = 15
PI = float(np.pi)

_cache = {}


def _build():
    import concourse.bacc as bacc
    import concourse.mybir as mybir
    import concourse.tile as tile
    from concourse import bass_isa

    dt = mybir.dt.float32
    AO = mybir.AluOpType
    AF = mybir.ActivationFunctionType

    nc = bacc.Bacc(None, target_bir_lowering=False, debug=True)

    # ---------------- I/O ----------------
    # per-point fields, host-replicated to [128, FR]:
    # 0 px, 1 py, 2 d0, 3 d1, 4 d2, 5 d3, 6 d4, 7 stride, 8 rlo, 9 rhi
    ptf = nc.dram_tensor("ptf", [10, 128, FR], dt, kind="ExternalInput")
    probsT = nc.dram_tensor("probsT", [NCLS, SH], dt, kind="ExternalInput")
    # per-partition scalars [128, 16]:
    # 0 qx, 1 qy, 2 w2, 3 h2, 4 a2r, 5 lab, 6 grpbase, 7 n0, 8 (32-g), 9 (g+1),
    # 10 isg0, 11 pad, 12-15 grp one-hot
    gsc = nc.dram_tensor("gsc", [128, 16], dt, kind="ExternalInput")
    glabrow = nc.dram_tensor("glabrow", [1, G], dt, kind="ExternalInput")

    out_labels = nc.dram_tensor("out_labels", [SH], mybir.dt.int32, kind="ExternalOutput")
    out_bt = nc.dram_tensor("out_bt", [SH, 4], dt, kind="ExternalOutput")
    out_at = nc.dram_tensor("out_at", [SH, 1], dt, kind="ExternalOutput")

    # collective bounce buffers
    g1in = nc.dram_tensor("g1in", [128, 40], dt)
    g1out = nc.dram_tensor("g1out", [NCORES * 128, 40], dt, addr_space="Shared")
    g2in = nc.dram_tensor("g2in", [128, 2], dt)
    g2out = nc.dram_tensor("g2out", [NCORES * 128, 2], dt, addr_space="Shared")

    dbg = {}

    with tile.TileContext(nc) as tc:
        with tc.tile_pool(name="main", bufs=1) as pool, \
             tc.tile_pool(name="psum", bufs=1, space="PSUM") as psum:

            def w(tag):
                return pool.tile([128, FR], dt, tag="w", bufs=55, name=tag)

            lw = w  # all wide tiles share one rotating arena

            def sm(tag, cols=1, dtype=dt, bufs=None):
                return pool.tile([128, cols], dtype, tag="sm%d" % cols,
                                 bufs=(bufs or 24), name=tag)

            V, S, GP, T = nc.vector, nc.scalar, nc.gpsimd, nc.tensor

            def tt(o, a, b, op):
                V.tensor_tensor(o, a, b, op=getattr(AO, op))

            def ts(o, a, s1, op0, s2=None, op1=None):
                if s2 is None:
                    V.tensor_scalar(o, a, s1, None, op0=getattr(AO, op0))
                else:
                    V.tensor_scalar(o, a, s1, s2, op0=getattr(AO, op0),
                                    op1=getattr(AO, op1))

            def stt(o, in0, sc, in1, op0, op1):
                V.scalar_tensor_tensor(o, in0, sc, in1,
                                       op0=getattr(AO, op0), op1=getattr(AO, op1))

            _consts = {}

            def constsc(val):
                if val not in _consts:
                    t = pool.tile([128, 1], dt, tag="cst", bufs=8,
                                  name="cst_%d" % len(_consts))
                    V.memset(t[:], val)
                    _consts[val] = t
                return _consts[val][:]

            def act(o, a, fn, bias=0.0, scale=1.0):
                if isinstance(bias, float) and bias != 0.0:
                    bias = constsc(bias)
                S.activation(o, a, getattr(AF, fn), bias=bias, scale=scale)

            # Cody-Waite split of 2*pi for range reduction (mod unsupported on HW)
            _tp = np.float64(2 * np.pi)
            _c1 = float(np.float32(6.28125))
            _c2 = float(np.float32(np.float32(_tp - _c1)))
            _c3 = float(np.float32(_tp - _c1 - np.float64(_c2)))

            def range_reduce(dst, src, shape):
                # dst = src - 2pi*round(src/2pi)  in [-pi-eps, pi+eps]
                tq = pool.tile(shape, dt, tag="rrq_%d" % shape[1], bufs=1, name="rrq")
                ts(tq[:], src, float(1.0 / _tp), "mult")
                ki = pool.tile(shape, mybir.dt.int32, tag="rri_%d" % shape[1], bufs=1, name="rri")
                V.tensor_copy(ki[:], tq[:])
                kf = pool.tile(shape, dt, tag="rrf_%d" % shape[1], bufs=1, name="rrf")
                V.tensor_copy(kf[:], ki[:])
                V.cody_waite_cascade(dst, src, kf[:], _c1, _c2, _c3)

            def blockmax(dst, src):
                # per-grp max over the 32 g-partitions; partition_all_reduce
                # requires base-partition-0 operands on HW, so bounce via ACT
                for grp in range(4):
                    bi = pool.tile([32, FR], dt, tag="br_in", bufs=2, name="br_in")
                    S.copy(bi[:], src[grp * 32:(grp + 1) * 32, :])
                    bo = pool.tile([32, FR], dt, tag="br_out", bufs=2, name="br_out")
                    GP.partition_all_reduce(bo[:], bi[:], channels=32,
                                            reduce_op=bass_isa.ReduceOp.max)
                    S.copy(dst[grp * 32:(grp + 1) * 32, :], bo[:])

            # ================= load scalars =================
            gs = pool.tile([128, 16], dt, tag="gs")
            nc.sync.dma_start(gs[:], gsc[:])
            qx, qy, w2, h2, a2r = (gs[:, i:i + 1] for i in range(5))
            labv, grpbase, n0sc, s32mg, gp1sc, isg0 = (gs[:, i:i + 1] for i in range(5, 11))

            sc = pool.tile([128, 24], dt, tag="sc")  # derived scalars
            col = [0]

            def newsc():
                c = col[0]
                col[0] += 1
                return sc[:, c:c + 1]

            a2 = newsc(); range_reduce(a2, a2r, [128, 1])
            s2 = newsc(); act(s2, a2, "Sin")
            a2w = newsc(); V.add_range_wrap(a2w, a2, PI / 2, PI, 2 * PI)
            c2 = newsc(); act(c2, a2w, "Sin")
            s2n = newsc(); ts(s2n, s2, -1.0, "mult")
            Wsc = newsc(); ts(Wsc, w2, 0.5, "mult")
            Hsc = newsc(); ts(Hsc, h2, 0.5, "mult")
            whp = newsc(); tt(whp, w2, h2, "mult")
            tmp1 = newsc(); tt(tmp1, qx, c2, "mult")
            tmp2 = newsc(); tt(tmp2, qy, s2, "mult")
            qoffx = newsc()
            tt(qoffx, tmp1, tmp2, "add"); ts(qoffx, qoffx, -1.0, "mult")
            tmp3 = newsc(); tt(tmp3, qx, s2, "mult")
            tmp4 = newsc(); tt(tmp4, qy, c2, "mult")
            qoffy = newsc(); tt(qoffy, tmp3, tmp4, "subtract")
            w2g = newsc(); ts(w2g, w2, 1e-10, "max")
            h2g = newsc(); ts(h2g, h2, 1e-10, "max")
            w2r = newsc(); V.reciprocal(w2r, w2g)
            w2r2 = newsc(); ts(w2r2, w2r, 2.0, "mult")
            h2r = newsc(); V.reciprocal(h2r, h2g)
            h2r2 = newsc(); ts(h2r2, h2r, 2.0, "mult")
            basesc = newsc(); tt(basesc, grpbase, n0sc, "add")

            # ================= load per-point replicated fields =================
            names = ["px", "py", "d0", "d1", "d2", "d3", "d4", "sS", "rlo", "rhi"]
            pt = {}
            for i, nm in enumerate(names):
                tile_ = w("in_" + nm)
                nc.sync.dma_start(tile_, ptf[i])
                pt[nm] = tile_

            # ================= per-point decode =================
            shalf = w("shalf"); ts(shalf, pt["sS"][:], 0.5, "mult")
            s15 = lw("s15"); ts(s15, pt["sS"][:], 1.5, "mult")
            t1_ = w("t1"); tt(t1_, pt["d0"][:], pt["d2"][:], "add")
            Araw = w("Araw"); tt(Araw, t1_, shalf, "mult")
            t2_ = w("t2"); tt(t2_, pt["d1"][:], pt["d3"][:], "add")
            Braw = w("Braw"); tt(Braw, t2_, shalf, "mult")
            Ag = lw("Ag"); ts(Ag, Araw, 1e-10, "max")
            Bg = lw("Bg"); ts(Bg, Braw, 1e-10, "max")
            t3_ = w("t3"); tt(t3_, pt["d2"][:], pt["d0"][:], "subtract")
            ot0 = w("ot0"); tt(ot0, t3_, shalf, "mult")
            t4_ = w("t4"); tt(t4_, pt["d3"][:], pt["d1"][:], "subtract")
            ot1 = w("ot1"); tt(ot1, t4_, shalf, "mult")
            th1 = w("th1"); range_reduce(th1, pt["d4"][:], [128, FR])
            s1 = lw("s1"); act(s1, th1, "Sin")
            th1w = w("th1w"); V.add_range_wrap(th1w, th1, PI / 2, PI, 2 * PI)
            c1 = lw("c1"); act(c1, th1w, "Sin")
            m1_ = w("m1"); tt(m1_, c1, ot0, "mult")
            m2_ = w("m2"); tt(m2_, s1, ot1, "mult")
            ox1 = w("ox1"); tt(ox1, m1_, m2_, "subtract")
            m3_ = w("m3"); tt(m3_, s1, ot0, "mult")
            m4_ = w("m4"); tt(m4_, c1, ot1, "mult")
            oy1 = w("oy1"); tt(oy1, m3_, m4_, "add")
            cx1 = lw("cx1"); tt(cx1, pt["px"][:], ox1, "add")
            cy1 = lw("cy1"); tt(cy1, pt["py"][:], oy1, "add")
            ab_ = w("ab"); tt(ab_, Ag, Bg, "mult")
            area1 = lw("area1"); ts(area1, ab_, 4.0, "mult")
            # local point index: iota + grpbase (+ n0 added later where needed)
            ioi = pool.tile([128, FR], mybir.dt.int32, tag="ioi")
            GP.iota(ioi[:], pattern=[[1, FR]], base=0, channel_multiplier=0)
            iof = w("iof"); V.tensor_copy(iof, ioi[:])
            idxloc = lw("idxloc"); V.tensor_scalar(idxloc, iof, grpbase, None, op0=AO.add)
            # recip stride (for bt output)
            ssg = w("ssg"); ts(ssg, pt["sS"][:], 1e-10, "max")
            rsS = lw("rsS")
            rs_scr = w("rs_scr")
            V.reciprocal_approx_accurate(rsS, ssg, rs_scr)

            # ================= dense pair quantities =================
            ox = lw("ox")
            V.tensor_scalar(ox, pt["px"][:], c2, qoffx, op0=AO.mult, op1=AO.add)
            stt(ox, pt["py"][:], s2, ox, "mult", "add")
            oy = lw("oy")
            V.tensor_scalar(oy, pt["px"][:], s2n, qoffy, op0=AO.mult, op1=AO.add)
            stt(oy, pt["py"][:], c2, oy, "mult", "add")
            l_ = lw("l_"); V.tensor_scalar(l_, ox, Wsc, None, op0=AO.add)
            r_ = lw("r_"); V.tensor_scalar(r_, ox, Wsc, -1.0, op0=AO.subtract, op1=AO.mult)
            tb = lw("tb"); V.tensor_scalar(tb, oy, Hsc, None, op0=AO.add)
            b_ = lw("b_"); V.tensor_scalar(b_, oy, Hsc, -1.0, op0=AO.subtract, op1=AO.mult)
            mnA = w("mnA"); tt(mnA, l_, tb, "min")
            mnB = w("mnB"); tt(mnB, r_, b_, "min")
            min4 = w("min4"); tt(min4, mnA, mnB, "min")
            mxA = w("mxA"); tt(mxA, l_, tb, "max")
            mxB = w("mxB"); tt(mxB, r_, b_, "max")
            maxrd = w("maxrd"); tt(maxrd, mxA, mxB, "max")
            rr1 = w("rr1"); tt(rr1, maxrd, pt["rlo"][:], "is_ge")
            rr2 = w("rr2"); tt(rr2, maxrd, pt["rhi"][:], "is_le")
            inrr = w("inrr"); tt(inrr, rr1, rr2, "logical_and")
            aox = w("aox"); act(aox, ox, "Abs")
            aoy = w("aoy"); act(aoy, oy, "Abs")
            ga = w("ga"); tt(ga, aox, s15, "is_lt")
            gb = w("gb"); tt(gb, aoy, s15, "is_lt")
            ins0 = w("ins0"); ts(ins0, min4, 0.0, "is_gt")
            va = w("va"); tt(va, ins0, ga, "logical_and")
            vb = w("vb"); tt(vb, va, gb, "logical_and")
            valid = lw("valid"); tt(valid, vb, inrr, "logical_and")
            ox2 = w("ox2"); V.tensor_scalar(ox2, ox, w2r2, None, op0=AO.mult)
            oy2 = w("oy2"); V.tensor_scalar(oy2, oy, h2r2, None, op0=AO.mult)
            sq1 = w("sq1"); act(sq1, ox2, "Square")
            sq2 = w("sq2"); act(sq2, oy2, "Square")
            zz = w("zz"); tt(zz, sq1, sq2, "add")
            sroot = w("sroot"); act(sroot, zz, "Sqrt", bias=5e-9, scale=0.5)
            cent = lw("cent"); act(cent, sroot, "Relu", bias=1.0, scale=-1.0)

            # ================= prob term (matmul over classes) =================
            # probs loaded as [128, FR]: partition (grp, cls padded to 32)
            p128 = pool.tile([128, FR], dt, tag="p128")
            V.memset(p128[:], 0.0)
            for grp in range(4):
                nc.sync.dma_start(p128[grp * 32:grp * 32 + NCLS, :],
                                  probsT[:, grp * FR:(grp + 1) * FR])
            e128 = pool.tile([128, FR], dt, tag="e128")
            act(e128[:], p128[:], "Exp")
            labr = pool.tile([1, G], dt, tag="labr")
            nc.sync.dma_start(labr[:], glabrow[:])
            labb = pool.tile([NCLS, G], dt, tag="labb")
            GP.partition_broadcast(labb[:], labr[:], channels=NCLS)
            clsio = pool.tile([NCLS, 1], mybir.dt.int32, tag="clsio")
            GP.iota(clsio[:], pattern=[[1, 1]], base=0, channel_multiplier=1)
            clsf = pool.tile([NCLS, 1], dt, tag="clsf")
            V.tensor_copy(clsf[:], clsio[:])
            onehotT = pool.tile([NCLS, G], dt, tag="onehotT")
            V.tensor_scalar(onehotT[:], labb[:], clsf[:], None, op0=AO.is_equal)
            # block-diagonal lhsT [128, 128] (32-padded class blocks)
            oh60 = pool.tile([128, 128], dt, tag="oh60")
            V.memset(oh60[:], 0.0)
            on60 = pool.tile([128, 128], dt, tag="on60")
            V.memset(on60[:], 0.0)
            for grp in range(4):
                V.tensor_copy(oh60[grp * 32:grp * 32 + NCLS,
                                   grp * 32:(grp + 1) * 32], onehotT[:])
                V.memset(on60[grp * 32:grp * 32 + NCLS,
                              grp * 32:(grp + 1) * 32], 1.0)

            HF = FR // 2  # 341
            Et = lw("Et"); Zt = lw("Zt")
            for h in range(2):
                psE = psum.tile([128, HF], dt, tag="psE", bufs=2, name="psE%d" % h)
                psZ = psum.tile([128, HF], dt, tag="psZ", bufs=2, name="psZ%d" % h)
                rs = e128[:, h * HF:(h + 1) * HF]
                T.matmul(psE[:], oh60[:], rs, start=True, stop=True)
                T.matmul(psZ[:], on60[:], rs, start=True, stop=True)
                S.copy(Et[:, h * HF:(h + 1) * HF], psE[:])
                S.copy(Zt[:, h * HF:(h + 1) * HF], psZ[:])
            rz = w("rz"); V.reciprocal(rz, Zt)
            probterm = lw("probterm")
            stt(probterm, Et, 0.6, rz, "mult", "mult")

            # ================= IoU =================
            u_ = lw("u_")
            V.tensor_scalar(u_, cx1, c2, qoffx, op0=AO.mult, op1=AO.add)
            stt(u_, cy1, s2, u_, "mult", "add")
            v_ = lw("v_")
            V.tensor_scalar(v_, cx1, s2n, qoffy, op0=AO.mult, op1=AO.add)
            stt(v_, cy1, c2, v_, "mult", "add")
            cp = lw("cp")
            V.tensor_scalar(cp, c1, c2, None, op0=AO.mult)
            stt(cp, s1, s2, cp, "mult", "add")
            sp = lw("sp")
            V.tensor_scalar(sp, c1, s2, None, op0=AO.mult)
            stt(sp, s1, c2, sp, "mult", "subtract")
            acp = w("acp"); act(acp, cp, "Abs")
            asp = w("asp"); act(asp, sp, "Abs")
            acpc = lw("acpc"); ts(acpc, acp, 1e-12, "max")
            aspc = lw("aspc"); ts(aspc, asp, 1e-12, "max")
            sgc = lw("sgc"); ts(sgc, cp, 0.0, "is_ge", 2.0, "mult")
            ts(sgc, sgc, 1.0, "subtract")
            sgs = lw("sgs"); ts(sgs, sp, 0.0, "is_ge", 2.0, "mult")
            ts(sgs, sgs, 1.0, "subtract")
            nsgc = lw("nsgc"); ts(nsgc, sgc, -1.0, "mult")
            nsgs = lw("nsgs"); ts(nsgs, sgs, -1.0, "mult")
            pp = lw("pp"); tt(pp, acpc, aspc, "mult")
            rpp = lw("rpp")
            rpp_scr = w("rpp_scr")
            V.reciprocal_approx_accurate(rpp, pp, rpp_scr)

            CK1 = lw("CK1"); CK2 = lw("CK2")
            k1a = w("k1a"); tt(k1a, u_, sp, "mult")
            k1b = w("k1b"); tt(k1b, v_, cp, "mult")
            tt(CK1, k1a, k1b, "subtract")
            k2a = w("k2a"); tt(k2a, u_, cp, "mult")
            k2b = w("k2b"); tt(k2b, v_, sp, "mult")
            tt(CK2, k2a, k2b, "add")

            Stot = lw("Stot")
            first_con = [True]

            def accum(conv):
                if first_con[0]:
                    V.tensor_copy(Stot, conv)
                    first_con[0] = False
                else:
                    tt(Stot, Stot, conv, "add")

            def clip_pass_det():
                # subject: det box (Ag,Bg) axes (cp,sp); clip vs [-W,W]x[-H,H]
                Acp = w("Acp"); tt(Acp, Ag, cp, "mult")
                Asp = w("Asp"); tt(Asp, Ag, sp, "mult")
                Bcp = w("Bcp"); tt(Bcp, Bg, cp, "mult")
                Bsp = w("Bsp"); tt(Bsp, Bg, sp, "mult")
                xp = w("xp"); tt(xp, u_, Acp, "add")
                xm = w("xm"); tt(xm, u_, Acp, "subtract")
                yp = w("yp"); tt(yp, v_, Asp, "add")
                ym = w("ym"); tt(ym, v_, Asp, "subtract")
                x0s, y0s = [], []
                for e, (bx, sx) in enumerate([(xp, "add"), (xp, "subtract"),
                                              (xm, "subtract"), (xm, "add")]):
                    x0 = w("x0_%d" % e); tt(x0, bx, Bsp, sx); x0s.append(x0)
                for e, (by, sy) in enumerate([(yp, "subtract"), (yp, "add"),
                                              (ym, "add"), (ym, "subtract")]):
                    y0 = w("y0_%d" % e); tt(y0, by, Bcp, sy); y0s.append(y0)
                # cE per edge
                cEs = []
                for e, (ck, base, sub) in enumerate([
                        (CK2, Ag, False), (CK1, Bg, True),
                        (CK2, Ag, True), (CK1, Bg, False)]):
                    cE = w("cE_%d" % e)
                    if sub:   # base - ck
                        tt(cE, base, ck, "subtract")
                    else:     # ck + base
                        tt(cE, ck, base, "add")
                    cEs.append(cE)
                # negL per family: -2B*pp, -2A*pp
                B2n = w("B2n"); ts(B2n, Bg, -2.0, "mult")
                A2n = w("A2n"); ts(A2n, Ag, -2.0, "mult")
                negL0 = w("negL0"); tt(negL0, pp, B2n, "mult")
                negL1 = w("negL1"); tt(negL1, pp, A2n, "mult")
                sgx = [nsgs, nsgc, sgs, sgc]
                sgy = [sgc, nsgs, nsgc, sgs]
                aux = [aspc, acpc, aspc, acpc]
                auy = [acpc, aspc, acpc, aspc]
                negL = [negL0, negL1, negL0, negL1]
                for e in range(4):
                    wx = w("wx"); tt(wx, x0s[e], sgx[e], "mult")
                    wy = w("wy"); tt(wy, y0s[e], sgy[e], "mult")
                    nhix = w("nhix"); stt(nhix, wx, Wsc, auy[e], "subtract", "mult")
                    nhiy = w("nhiy"); stt(nhiy, wy, Hsc, aux[e], "subtract", "mult")
                    loxn = w("loxn"); stt(loxn, wx, Wsc, auy[e], "add", "mult")
                    loyn = w("loyn"); stt(loyn, wy, Hsc, aux[e], "add", "mult")
                    nt1a = w("nt1a"); tt(nt1a, nhix, nhiy, "max")
                    nt1 = w("nt1"); tt(nt1, nt1a, negL[e], "max")
                    mna = w("mna"); tt(mna, loxn, loyn, "min")
                    mn0 = w("mn0"); ts(mn0, mna, 0.0, "min")
                    dtr = w("dtr"); tt(dtr, mn0, nt1, "subtract")
                    dtp = w("dtp"); act(dtp, dtr, "Relu")
                    con = w("con"); tt(con, dtp, cEs[e], "mult")
                    accum(con)

            def clip_pass_gt():
                # subject: gt box (W,H) axes (cp,-sp) at (up,vp)=(-CK2,CK1);
                # clip vs [-Ag,Ag]x[-Bg,Bg]; corrections folded into cE.
                up = w("up"); ts(up, CK2, -1.0, "mult")
                vp = CK1
                # CKg1 = up*cp - vp*sp ; CKg2 = up*sp + vp*cp
                CKg1 = w("CKg1"); CKg2 = w("CKg2")
                g1a = w("g1a"); tt(g1a, up, cp, "mult")
                g1b = w("g1b"); tt(g1b, vp, sp, "mult")
                tt(CKg1, g1a, g1b, "subtract")
                g2a = w("g2a"); tt(g2a, up, sp, "mult")
                g2b = w("g2b"); tt(g2b, vp, cp, "mult")
                tt(CKg2, g2a, g2b, "add")
                # corners with sin=-sp: Asp_g = W*(-sp) etc (W,H scalars)
                Wcp = w("Wcp"); V.tensor_scalar(Wcp, cp, Wsc, None, op0=AO.mult)
                Wspn = w("Wspn"); V.tensor_scalar(Wspn, sp, Wsc, -1.0, op0=AO.mult, op1=AO.mult)
                Hcp = w("Hcp"); V.tensor_scalar(Hcp, cp, Hsc, None, op0=AO.mult)
                Hspn = w("Hspn"); V.tensor_scalar(Hspn, sp, Hsc, -1.0, op0=AO.mult, op1=AO.mult)
                xp = w("xpg"); tt(xp, up, Wcp, "add")
                xm = w("xmg"); tt(xm, up, Wcp, "subtract")
                yp = w("ypg"); tt(yp, vp, Wspn, "add")
                ym = w("ymg"); tt(ym, vp, Wspn, "subtract")
                x0s, y0s = [], []
                for e, (bx, sx) in enumerate([(xp, "add"), (xp, "subtract"),
                                              (xm, "subtract"), (xm, "add")]):
                    x0 = w("gx0_%d" % e); tt(x0, bx, Hspn, sx); x0s.append(x0)
                for e, (by, sy) in enumerate([(yp, "subtract"), (yp, "add"),
                                              (ym, "add"), (ym, "subtract")]):
                    y0 = w("gy0_%d" % e); tt(y0, by, Hcp, sy); y0s.append(y0)
                # cE with corrections: [CKg1+W+u, CKg2+H+v, -(CKg1-W+u), -(CKg2-H+v)]
                cEs = []
                for e, (ck, ssc, uv, neg) in enumerate([
                        (CKg1, Wsc, u_, False), (CKg2, Hsc, v_, False),
                        (CKg1, Wsc, u_, True), (CKg2, Hsc, v_, True)]):
                    cE = w("gcE_%d" % e)
                    if neg:
                        stt(cE, ck, ssc, uv, "subtract", "add")   # ck-W+u (negated later)
                    else:
                        stt(cE, ck, ssc, uv, "add", "add")        # ck+W+u
                    cEs.append(cE)
                # negL: -2H*pp, -2W*pp (scalars -h2, -w2 times pp)
                h2n = w("h2n"); V.tensor_scalar(h2n, pp, h2, -1.0, op0=AO.mult, op1=AO.mult)
                w2n = w("w2n"); V.tensor_scalar(w2n, pp, w2, -1.0, op0=AO.mult, op1=AO.mult)
                sgx = [sgs, nsgc, nsgs, sgc]
                sgy = [sgc, sgs, nsgc, nsgs]
                aux = [aspc, acpc, aspc, acpc]
                auy = [acpc, aspc, acpc, aspc]
                negL = [h2n, w2n, h2n, w2n]
                for e in range(4):
                    wx = w("gwx"); tt(wx, x0s[e], sgx[e], "mult")
                    wy = w("gwy"); tt(wy, y0s[e], sgy[e], "mult")
                    nhix = w("gnhix"); tt(nhix, wx, Ag, "subtract"); tt(nhix, nhix, auy[e], "mult")
                    nhiy = w("gnhiy"); tt(nhiy, wy, Bg, "subtract"); tt(nhiy, nhiy, aux[e], "mult")
                    loxn = w("gloxn"); tt(loxn, wx, Ag, "add"); tt(loxn, loxn, auy[e], "mult")
                    loyn = w("gloyn"); tt(loyn, wy, Bg, "add"); tt(loyn, loyn, aux[e], "mult")
                    nt1a = w("gnt1a"); tt(nt1a, nhix, nhiy, "max")
                    nt1 = w("gnt1"); tt(nt1, nt1a, negL[e], "max")
                    mna = w("gmna"); tt(mna, loxn, loyn, "min")
                    mn0 = w("gmn0"); ts(mn0, mna, 0.0, "min")
                    dtr = w("gdtr"); tt(dtr, mn0, nt1, "subtract")
                    dtp = w("gdtp"); act(dtp, dtr, "Relu")
                    con = w("gcon"); tt(con, dtp, cEs[e], "mult")
                    if e >= 2:
                        tt(Stot, Stot, con, "subtract")
                    else:
                        accum(con)

            clip_pass_det()
            clip_pass_gt()

            inter = w("inter")
            ts(inter, Stot, 0.5, "mult")
            tt(inter, inter, rpp, "mult")
            un = w("un")
            V.tensor_scalar(un, area1, whp, None, op0=AO.add)
            tt(un, un, inter, "subtract")
            ts(un, un, 1e-8, "max")
            run_ = w("run")
            run_scr = w("run_scr")
            V.reciprocal_approx_accurate(run_, un, run_scr)
            iou = lw("iou"); tt(iou, inter, run_, "mult")

            # ================= cost & per-core topk =================
            cost = lw("cost")
            stt(cost, cent, 0.2, probterm, "mult", "add")
            stt(cost, iou, 0.2, cost, "mult", "add")
            tt(cost, cost, valid, "mult")

            mx1 = sm("mx1", 8); mi1 = pool.tile([128, 8], mybir.dt.uint32, tag="mi1")
            V.max_with_indices(mx1[:, 0:8], mi1[:], cost)
            costc = w("costc")
            V.match_replace(costc, mx1[:, 0:8], cost, -1e30)
            mx2 = sm("mx2", 8); mi2 = pool.tile([128, 8], mybir.dt.uint32, tag="mi2")
            V.max_with_indices(mx2[:, 0:8], mi2[:], costc)
            gi1 = sm("gi1", 8); V.tensor_copy(gi1[:], mi1[:])
            V.tensor_scalar(gi1[:], gi1[:], basesc, None, op0=AO.add)
            gi2 = sm("gi2", 8); V.tensor_copy(gi2[:], mi2[:])
            V.tensor_scalar(gi2[:], gi2[:], basesc, None, op0=AO.add)
            cmx = sm("cmx", 8); cmi = pool.tile([128, 8], mybir.dt.uint32, tag="cmi")
            V.max_with_indices(cmx[:, 0:8], cmi[:], cent)
            cgi = sm("cgi", 8); V.tensor_copy(cgi[:], cmi[:])
            V.tensor_scalar(cgi[:], cgi[:], basesc, None, op0=AO.add)

            pay = pool.tile([128, 40], dt, tag="pay")
            V.tensor_copy(pay[:, 0:8], mx1[:, 0:8])
            V.tensor_copy(pay[:, 8:16], mx2[:, 0:8])
            V.tensor_copy(pay[:, 16:24], gi1[:])
            V.tensor_copy(pay[:, 24:32], gi2[:])
            V.tensor_copy(pay[:, 32:33], cmx[:, 0:1])
            V.tensor_copy(pay[:, 33:34], cgi[:, 0:1])
            V.memset(pay[:, 34:40], 0.0)
            GP.dma_start(g1in[:], pay[:])
            GP.collective_compute(
                "AllGather", mybir.AluOpType.bypass,
                replica_groups=[list(range(NCORES))],
                ins=[g1in[:]], outs=[g1out[:]])

            # reload: per partition (grp,g) -> g's candidates from all (core,grp')
            g1v = g1out[:].rearrange("(c p) k -> c p k", c=NCORES)  # [8,128,40]
            # vals [128, 8*4*16]; same [32, 512] content loaded to each grp block
            vals = pool.tile([128, 512], dt, tag="vals")
            src = g1v[:, :, 0:16].rearrange("c (a b) k -> b c a k", a=4)  # [32,8,4,16]
            for x in range(4):
                nc.sync.dma_start(
                    vals[x * 32:(x + 1) * 32, :].rearrange(
                        "p (c a k) -> p c a k", c=NCORES, a=4), src)
            vmx1 = sm("vmx1", 8); V.max(vmx1[:, 0:8], vals[:])
            valsc = pool.tile([128, 512], dt, tag="valsc")
            V.match_replace(valsc[:], vmx1[:, 0:8], vals[:], -1e30)
            vmx2 = sm("vmx2", 8); V.max(vmx2[:, 0:8], valsc[:])
            thr = sm("thr"); V.tensor_copy(thr[:], vmx2[:, 6:7])  # 15th largest
            threff = sm("threff"); ts(threff[:], thr[:], 1e-20, "max")

            # cent global argmax: vals at col 32, idx at col 33
            cvals = pool.tile([128, 32], dt, tag="cvals")
            srcv = g1v[:, :, 32:33].rearrange("c (a b) k -> b (c a k)", a=4)
            cidxs = pool.tile([128, 32], dt, tag="cidxs")
            srci = g1v[:, :, 33:34].rearrange("c (a b) k -> b (c a k)", a=4)
            for x in range(4):
                nc.sync.dma_start(cvals[x * 32:(x + 1) * 32, :], srcv)
                nc.sync.dma_start(cidxs[x * 32:(x + 1) * 32, :], srci)
            cm_ = sm("cm_")
            V.tensor_reduce(cm_[:], cvals[:], axis=mybir.AxisListType.X, op=AO.max)
            ceq = pool.tile([128, 32], dt, tag="ceq")
            V.tensor_scalar(ceq[:], cvals[:], cm_[:], None, op0=AO.is_equal)
            cbig = pool.tile([128, 32], dt, tag="cbig")
            ts(cbig[:], ceq[:], -1e9, "mult", 1e9, "add")  # (1-eq)*1e9
            cnd = pool.tile([128, 32], dt, tag="cnd")
            tt(cnd[:], ceq[:], cidxs[:], "mult")
            tt(cnd[:], cnd[:], cbig[:], "add")
            anch = sm("anch")
            V.tensor_reduce(anch[:], cnd[:], axis=mybir.AxisListType.X, op=AO.min)
            anchloc = sm("anchloc")
            V.tensor_scalar(anchloc[:], anch[:], n0sc, None, op0=AO.subtract)

            # ================= selection tail =================
            sel = w("sel")
            V.tensor_scalar(sel, cost, threff[:], None, op0=AO.is_ge)
            areas = lw("areas")
            V.tensor_scalar(areas, sel, whp, None, op0=AO.mult)
            amax = lw("amax")
            blockmax(amax, areas)
            eq = w("eq"); tt(eq, areas, amax, "is_equal")
            gq = lw("gq")
            V.tensor_scalar(gq, eq, s32mg, 1.0, op0=AO.mult, op1=AO.subtract)
            gqm = lw("gqm")
            blockmax(gqm, gq)
            oh = lw("oh"); tt(oh, gq, gqm, "is_equal")
            matchedT = lw("matchedT"); ts(matchedT, amax, 0.0, "is_gt")
            ohm = lw("ohm"); tt(ohm, oh, matchedT, "mult")
            cntp = sm("cntp")
            V.tensor_reduce(cntp[:], ohm, axis=mybir.AxisListType.X, op=AO.add)
            pay2 = pool.tile([128, 2], dt, tag="pay2")
            V.tensor_copy(pay2[:, 0:1], cntp[:])
            V.memset(pay2[:, 1:2], 0.0)
            GP.dma_start(g2in[:], pay2[:])
            GP.collective_compute(
                "AllGather", mybir.AluOpType.bypass,
                replica_groups=[list(range(NCORES))],
                ins=[g2in[:]], outs=[g2out[:]])
            g2v = g2out[:].rearrange("(c p) k -> c p k", c=NCORES)
            cnt_all = pool.tile([128, 32], dt, tag="cnt_all")
            srcc = g2v[:, :, 0:1].rearrange("c (a b) k -> b (c a k)", a=4)
            for x in range(4):
                nc.sync.dma_start(cnt_all[x * 32:(x + 1) * 32, :], srcc)
            cnts = sm("cnts")
            V.tensor_reduce(cnts[:], cnt_all[:], axis=mybir.AxisListType.X, op=AO.add)
            cnt0 = sm("cnt0"); ts(cnt0[:], cnts[:], 0.0, "is_equal")

            # ================= rematch + final one-hot =================
            fix = w("fix")
            V.tensor_scalar(fix, idxloc, anchloc[:], cnt0[:],
                            op0=AO.is_equal, op1=AO.logical_and)
            fixg = w("fixg")
            V.tensor_scalar(fixg, fix, gp1sc, None, op0=AO.mult)
            fgm = lw("fgm")
            blockmax(fgm, fixg)
            anyfixT = w("anyfixT"); ts(anyfixT, fgm, 0.0, "is_gt")
            fixu = w("fixu"); tt(fixu, fixg, fgm, "is_equal")
            fixf = w("fixf"); tt(fixf, fixu, anyfixT, "mult")
            nanf = w("nanf"); ts(nanf, anyfixT, -1.0, "mult", 1.0, "add")
            ohm2 = w("ohm2"); tt(ohm2, ohm, nanf, "mult")
            ohf = lw("ohf"); tt(ohf, ohm2, fixf, "add")
            # default g0 one-hot for unassigned points
            notm = w("notm"); ts(notm, matchedT, -1.0, "mult", 1.0, "add")
            dm = lw("dm"); tt(dm, notm, nanf, "mult")
            dmg0 = w("dmg0")
            V.tensor_scalar(dmg0, dm, isg0, None, op0=AO.mult)
            ohd = lw("ohd"); tt(ohd, ohf, dmg0, "add")

            # ================= gathers (matmul over g) + outputs =================
            HF2 = HF  # 341
            ohd_rs = lw("ohd_rs"); tt(ohd_rs, ohd, rsS, "mult")  # pre-scaled by 1/stride
            prods = []
            for nm, srcT in [("l", l_), ("t", tb), ("r", r_), ("b", b_)]:
                p_ = lw("p_" + nm); tt(p_, ohd_rs, srcT, "mult")
                prods.append((nm, p_))
            grpOH = gs[:, 12:16]                 # [128, 4] grp one-hot selector
            lab4 = pool.tile([128, 4], dt, tag="lab4")
            V.tensor_scalar(lab4[:], grpOH, labv, None, op0=AO.mult)
            at4 = pool.tile([128, 4], dt, tag="at4")
            V.tensor_scalar(at4[:], grpOH, a2r, None, op0=AO.mult)
            lab15 = sm("lab15")
            ts(lab15[:], labv, -1.0, "mult", 15.0, "add")     # 15 - lab_g
            lab4f = pool.tile([128, 4], dt, tag="lab4f")
            V.tensor_scalar(lab4f[:], grpOH, lab15[:], None, op0=AO.mult)

            # quantities to gather: (name, lhsT [128,4], rhs wide tile)
            quants = [("lab", lab4[:], ohd), ("labf", lab4f[:], dmg0),
                      ("at", at4[:], ohd)]
            for nm, p_ in prods:
                quants.append(("q" + nm, grpOH, p_))

            gout = {}
            for qi, (nm, lhsT, rhsT) in enumerate(quants):
                res = pool.tile([4, FR], dt, tag="g_" + nm, name="g_" + nm)
                for h in range(2):
                    ps_ = psum.tile([4, HF2], dt, tag="gps", bufs=4,
                                    name="gps_%s_%d" % (nm, h))
                    T.matmul(ps_[:], lhsT,
                             rhsT[:, h * HF2:(h + 1) * HF2],
                             start=True, stop=True)
                    S.copy(res[:, h * HF2:(h + 1) * HF2], ps_[:])
                gout[nm] = res

            # labels = gather(ohf*lab) + gather(dmg0*(15-lab))
            labf = pool.tile([4, FR], dt, tag="labf")
            tt(labf[:], gout["lab"][:], gout["labf"][:], "add")
            labi = pool.tile([4, FR], mybir.dt.int32, tag="labi")
            V.tensor_copy(labi[:], labf[:])
            nc.sync.dma_start(out_labels[:].rearrange("(a f) -> a f", a=4), labi[:])

            for k, nm in enumerate(["ql", "qt", "qr", "qb"]):
                dstv = out_bt[:, k:k + 1].rearrange("(a f) o -> a (f o)", a=4)
                nc.sync.dma_start(dstv, gout[nm][:])
            dsta = out_at[:, 0:1].rearrange("(a f) o -> a (f o)", a=4)
            nc.sync.dma_start(dsta, gout["at"][:])

    nc.compile()
    return nc, dbg


def _prep_inputs(inputs):
    """Shard + replicate host-side (pure layout work)."""
    f32 = np.float32
    points = np.asarray(inputs["points"], f32)
    rr = np.asarray(inputs["regress_ranges"], f32)
    spp = np.asarray(inputs["stride_pp"], f32)
    gtb = np.asarray(inputs["gt_bboxes"], f32)
    gtl = np.asarray(inputs["gt_labels"])
    bp = np.asarray(inputs["bbox_preds"], f32)
    probs = np.asarray(inputs["probs"], f32)

    fields = [points[:, 0], points[:, 1], bp[:, 0], bp[:, 1], bp[:, 2],
              bp[:, 3], bp[:, 4], spp, rr[:, 0], rr[:, 1]]
    probsT_full = np.ascontiguousarray(probs.T)  # [15, N]

    g_idx = np.arange(128) % 32
    grp_idx = np.arange(128) // 32
    gsc_base = np.zeros((128, 16), f32)
    gsc_base[:, 0] = gtb[g_idx, 0]
    gsc_base[:, 1] = gtb[g_idx, 1]
    gsc_base[:, 2] = gtb[g_idx, 2]
    gsc_base[:, 3] = gtb[g_idx, 3]
    gsc_base[:, 4] = gtb[g_idx, 4]
    gsc_base[:, 5] = gtl[g_idx].astype(f32)
    gsc_base[:, 6] = (grp_idx * FR).astype(f32)
    gsc_base[:, 8] = (32 - g_idx).astype(f32)
    gsc_base[:, 9] = (g_idx + 1).astype(f32)
    gsc_base[:, 10] = (g_idx == 0).astype(f32)
    for grp in range(4):
        gsc_base[:, 12 + grp] = (grp_idx == grp).astype(f32)
    glabrow = gtl.astype(f32).reshape(1, G)

    in_maps = []
    for c in range(NCORES):
        sl = slice(c * SH, (c + 1) * SH)
        ptf = np.empty((10, 128, FR), f32)
        for i, arr in enumerate(fields):
            shard = arr[sl].reshape(4, 1, FR)
            ptf[i] = np.broadcast_to(shard, (4, 32, FR)).reshape(128, FR)
        gsc = gsc_base.copy()
        gsc[:, 7] = c * SH
        in_maps.append({
            "ptf": ptf,
            "probsT": np.ascontiguousarray(probsT_full[:, sl]),
            "gsc": gsc,
            "glabrow": glabrow,
        })
    return in_maps


def kernel(**inputs):
    from concourse.bass_utils import run_bass_kernel_spmd
    if "nc" not in _cache:
        _cache["nc"], _ = _build()
    nc = _cache["nc"]
    in_maps = _prep_inputs(inputs)
    res = run_bass_kernel_spmd(nc, in_maps, list(range(NCORES)))
    labels = np.concatenate([res.results[c]["out_labels"] for c in range(NCORES)])
    bt = np.concatenate([res.results[c]["out_bt"] for c in range(NCORES)])
    at = np.concatenate([res.results[c]["out_at"] for c in range(NCORES)])
    return labels.astype(np.int32), bt, at


# revision 31
# speedup vs baseline: 2.6201x; 2.6201x over previous
"""OBB label assigner on 8 Trainium2 NeuronCores (Bass/Tile SPMD kernel).

Self-contained: builds, compiles and runs the kernel; host code only
shards/replicates input layouts and concatenates outputs.

Layout per core ("layout B"): SBUF tiles [128, 682] f32 where
partition p = grp*32 + g (grp in 0..3 selects a 682-point block of the
core's 2728-point shard; g in 0..31 is the GT index). Per-GT constants are
per-partition scalars; per-point fields are replicated across the 32 g
partitions of each grp block (host prepares the replicated layout).

Rotated-box IoU: sort-free Green's-theorem formulation. Each box's 4 edges
are clipped against the other box's rect (in that box's frame) with a
scaled Liang-Barsky parameterization (no divisions), and the line-integral
cross terms are accumulated with a translation correction for the
second frame. Validated against the reference in proto2.py.
"""
import numpy as np

N = 21824
NCORES = 8
SH = N // NCORES          # 2728 points per core
FR = SH // 4              # 682 free width
G = 32
NCLS = 15
BG = 15
PI = float(np.pi)

_cache = {}


def _build():
    import concourse.bacc as bacc
    import concourse.mybir as mybir
    import concourse.tile as tile
    from concourse import bass_isa

    dt = mybir.dt.float32
    AO = mybir.AluOpType
    AF = mybir.ActivationFunctionType

    nc = bacc.Bacc(None, target_bir_lowering=False, debug=True)

    # ---------------- I/O ----------------
    # per-point fields, host-replicated to [128, FR]:
    # 0 px, 1 py, 2 d0, 3 d1, 4 d2, 5 d3, 6 d4, 7 stride, 8 rlo, 9 rhi
    ptf = nc.dram_tensor("ptf", [10, SH], dt, kind="ExternalInput")
    probsT = nc.dram_tensor("probsT", [NCLS, SH], dt, kind="ExternalInput")
    # per-partition scalars [128, 16]:
    # 0 qx, 1 qy, 2 w2, 3 h2, 4 a2r, 5 lab, 6 grpbase, 7 n0, 8 (32-g), 9 (g+1),
    # 10 isg0, 11 pad, 12-15 grp one-hot
    gsc = nc.dram_tensor("gsc", [128, 16], dt, kind="ExternalInput")
    glabrow = nc.dram_tensor("glabrow", [1, G], dt, kind="ExternalInput")

    out_labels = nc.dram_tensor("out_labels", [SH], mybir.dt.int32, kind="ExternalOutput")
    out_bt = nc.dram_tensor("out_bt", [SH, 4], dt, kind="ExternalOutput")
    out_at = nc.dram_tensor("out_at", [SH, 1], dt, kind="ExternalOutput")

    # collective bounce buffers
    g1in = nc.dram_tensor("g1in", [128, 40], dt)
    g1out = nc.dram_tensor("g1out", [NCORES * 128, 40], dt, addr_space="Shared")
    g2in = nc.dram_tensor("g2in", [128, 2], dt)
    g2out = nc.dram_tensor("g2out", [NCORES * 128, 2], dt, addr_space="Shared")

    dbg = {}

    with tile.TileContext(nc) as tc:
        with tc.tile_pool(name="main", bufs=1) as pool, \
             tc.tile_pool(name="psum", bufs=1, space="PSUM") as psum:

            def w(tag):
                return pool.tile([128, FR], dt, tag="w", bufs=55, name=tag)

            lw = w  # all wide tiles share one rotating arena

            def sm(tag, cols=1, dtype=dt, bufs=None):
                return pool.tile([128, cols], dtype, tag="sm%d" % cols,
                                 bufs=(bufs or 24), name=tag)

            V, S, GP, T = nc.vector, nc.scalar, nc.gpsimd, nc.tensor

            def tt(o, a, b, op):
                V.tensor_tensor(o, a, b, op=getattr(AO, op))

            def ts(o, a, s1, op0, s2=None, op1=None):
                if s2 is None:
                    V.tensor_scalar(o, a, s1, None, op0=getattr(AO, op0))
                else:
                    V.tensor_scalar(o, a, s1, s2, op0=getattr(AO, op0),
                                    op1=getattr(AO, op1))

            def stt(o, in0, sc, in1, op0, op1):
                V.scalar_tensor_tensor(o, in0, sc, in1,
                                       op0=getattr(AO, op0), op1=getattr(AO, op1))

            _consts = {}

            def constsc(val):
                if val not in _consts:
                    t = pool.tile([128, 1], dt, tag="cst", bufs=8,
                                  name="cst_%d" % len(_consts))
                    V.memset(t[:], val)
                    _consts[val] = t
                return _consts[val][:]

            def act(o, a, fn, bias=0.0, scale=1.0):
                if isinstance(bias, float) and bias != 0.0:
                    bias = constsc(bias)
                S.activation(o, a, getattr(AF, fn), bias=bias, scale=scale)

            # Cody-Waite split of 2*pi for range reduction (mod unsupported on HW)
            _tp = np.float64(2 * np.pi)
            _c1 = float(np.float32(6.28125))
            _c2 = float(np.float32(np.float32(_tp - _c1)))
            _c3 = float(np.float32(_tp - _c1 - np.float64(_c2)))

            def range_reduce(dst, src, shape):
                # dst = src - 2pi*round(src/2pi)  in [-pi-eps, pi+eps]
                tq = pool.tile(shape, dt, tag="rrq_%d" % shape[1], bufs=1, name="rrq")
                ts(tq[:], src, float(1.0 / _tp), "mult")
                ki = pool.tile(shape, mybir.dt.int32, tag="rri_%d" % shape[1], bufs=1, name="rri")
                V.tensor_copy(ki[:], tq[:])
                kf = pool.tile(shape, dt, tag="rrf_%d" % shape[1], bufs=1, name="rrf")
                V.tensor_copy(kf[:], ki[:])
                V.cody_waite_cascade(dst, src, kf[:], _c1, _c2, _c3)

            def blockmax(dst, src):
                # per-grp max over the 32 g-partitions; partition_all_reduce
                # requires base-partition-0 operands on HW, so bounce via ACT
                for grp in range(4):
                    bi = pool.tile([32, FR], dt, tag="br_in", bufs=2, name="br_in")
                    S.copy(bi[:], src[grp * 32:(grp + 1) * 32, :])
                    bo = pool.tile([32, FR], dt, tag="br_out", bufs=2, name="br_out")
                    GP.partition_all_reduce(bo[:], bi[:], channels=32,
                                            reduce_op=bass_isa.ReduceOp.max)
                    S.copy(dst[grp * 32:(grp + 1) * 32, :], bo[:])

            # ================= load scalars =================
            gs = pool.tile([128, 16], dt, tag="gs")
            nc.sync.dma_start(gs[:], gsc[:])
            qx, qy, w2, h2, a2r = (gs[:, i:i + 1] for i in range(5))
            labv, grpbase, n0sc, s32mg, gp1sc, isg0 = (gs[:, i:i + 1] for i in range(5, 11))

            sc = pool.tile([128, 24], dt, tag="sc")  # derived scalars
            col = [0]

            def newsc():
                c = col[0]
                col[0] += 1
                return sc[:, c:c + 1]

            a2 = newsc(); range_reduce(a2, a2r, [128, 1])
            s2 = newsc(); act(s2, a2, "Sin")
            a2w = newsc(); V.add_range_wrap(a2w, a2, PI / 2, PI, 2 * PI)
            c2 = newsc(); act(c2, a2w, "Sin")
            s2n = newsc(); ts(s2n, s2, -1.0, "mult")
            Wsc = newsc(); ts(Wsc, w2, 0.5, "mult")
            Hsc = newsc(); ts(Hsc, h2, 0.5, "mult")
            whp = newsc(); tt(whp, w2, h2, "mult")
            tmp1 = newsc(); tt(tmp1, qx, c2, "mult")
            tmp2 = newsc(); tt(tmp2, qy, s2, "mult")
            qoffx = newsc()
            tt(qoffx, tmp1, tmp2, "add"); ts(qoffx, qoffx, -1.0, "mult")
            tmp3 = newsc(); tt(tmp3, qx, s2, "mult")
            tmp4 = newsc(); tt(tmp4, qy, c2, "mult")
            qoffy = newsc(); tt(qoffy, tmp3, tmp4, "subtract")
            w2g = newsc(); ts(w2g, w2, 1e-10, "max")
            h2g = newsc(); ts(h2g, h2, 1e-10, "max")
            w2r = newsc(); V.reciprocal(w2r, w2g)
            w2r2 = newsc(); ts(w2r2, w2r, 2.0, "mult")
            h2r = newsc(); V.reciprocal(h2r, h2g)
            h2r2 = newsc(); ts(h2r2, h2r, 2.0, "mult")
            basesc = newsc(); tt(basesc, grpbase, n0sc, "add")

            # ================= load per-point replicated fields =================
            names = ["px", "py", "d0", "d1", "d2", "d3", "d4", "sS", "rlo", "rhi"]
            pt = {}
            for i, nm in enumerate(names):
                tile_ = w("in_" + nm)
                for grp in range(4):
                    srcg = ptf[i, grp * FR:(grp + 1) * FR]
                    srcg = srcg.unsqueeze(0).broadcast_to([32, FR])
                    nc.sync.dma_start(tile_[grp * 32:(grp + 1) * 32, :], srcg)
                pt[nm] = tile_

            # ================= per-point decode =================
            shalf = w("shalf"); ts(shalf, pt["sS"][:], 0.5, "mult")
            s15 = lw("s15"); ts(s15, pt["sS"][:], 1.5, "mult")
            t1_ = w("t1"); tt(t1_, pt["d0"][:], pt["d2"][:], "add")
            Araw = w("Araw"); tt(Araw, t1_, shalf, "mult")
            t2_ = w("t2"); tt(t2_, pt["d1"][:], pt["d3"][:], "add")
            Braw = w("Braw"); tt(Braw, t2_, shalf, "mult")
            Ag = lw("Ag"); ts(Ag, Araw, 1e-10, "max")
            Bg = lw("Bg"); ts(Bg, Braw, 1e-10, "max")
            t3_ = w("t3"); tt(t3_, pt["d2"][:], pt["d0"][:], "subtract")
            ot0 = w("ot0"); tt(ot0, t3_, shalf, "mult")
            t4_ = w("t4"); tt(t4_, pt["d3"][:], pt["d1"][:], "subtract")
            ot1 = w("ot1"); tt(ot1, t4_, shalf, "mult")
            th1 = w("th1"); range_reduce(th1, pt["d4"][:], [128, FR])
            s1 = lw("s1"); act(s1, th1, "Sin")
            th1w = w("th1w"); V.add_range_wrap(th1w, th1, PI / 2, PI, 2 * PI)
            c1 = lw("c1"); act(c1, th1w, "Sin")
            m1_ = w("m1"); tt(m1_, c1, ot0, "mult")
            m2_ = w("m2"); tt(m2_, s1, ot1, "mult")
            ox1 = w("ox1"); tt(ox1, m1_, m2_, "subtract")
            m3_ = w("m3"); tt(m3_, s1, ot0, "mult")
            m4_ = w("m4"); tt(m4_, c1, ot1, "mult")
            oy1 = w("oy1"); tt(oy1, m3_, m4_, "add")
            cx1 = lw("cx1"); tt(cx1, pt["px"][:], ox1, "add")
            cy1 = lw("cy1"); tt(cy1, pt["py"][:], oy1, "add")
            ab_ = w("ab"); tt(ab_, Ag, Bg, "mult")
            area1 = lw("area1"); ts(area1, ab_, 4.0, "mult")
            # local point index: iota + grpbase (+ n0 added later where needed)
            ioi = pool.tile([128, FR], mybir.dt.int32, tag="ioi")
            GP.iota(ioi[:], pattern=[[1, FR]], base=0, channel_multiplier=0)
            iof = w("iof"); V.tensor_copy(iof, ioi[:])
            idxloc = lw("idxloc"); V.tensor_scalar(idxloc, iof, grpbase, None, op0=AO.add)
            # recip stride (for bt output)
            ssg = w("ssg"); ts(ssg, pt["sS"][:], 1e-10, "max")
            rsS = lw("rsS")
            rs_scr = w("rs_scr")
            V.reciprocal_approx_accurate(rsS, ssg, rs_scr)

            # ================= dense pair quantities =================
            ox = lw("ox")
            V.tensor_scalar(ox, pt["px"][:], c2, qoffx, op0=AO.mult, op1=AO.add)
            stt(ox, pt["py"][:], s2, ox, "mult", "add")
            oy = lw("oy")
            V.tensor_scalar(oy, pt["px"][:], s2n, qoffy, op0=AO.mult, op1=AO.add)
            stt(oy, pt["py"][:], c2, oy, "mult", "add")
            l_ = lw("l_"); V.tensor_scalar(l_, ox, Wsc, None, op0=AO.add)
            r_ = lw("r_"); V.tensor_scalar(r_, ox, Wsc, -1.0, op0=AO.subtract, op1=AO.mult)
            tb = lw("tb"); V.tensor_scalar(tb, oy, Hsc, None, op0=AO.add)
            b_ = lw("b_"); V.tensor_scalar(b_, oy, Hsc, -1.0, op0=AO.subtract, op1=AO.mult)
            mnA = w("mnA"); tt(mnA, l_, tb, "min")
            mnB = w("mnB"); tt(mnB, r_, b_, "min")
            min4 = w("min4"); tt(min4, mnA, mnB, "min")
            mxA = w("mxA"); tt(mxA, l_, tb, "max")
            mxB = w("mxB"); tt(mxB, r_, b_, "max")
            maxrd = w("maxrd"); tt(maxrd, mxA, mxB, "max")
            rr1 = w("rr1"); tt(rr1, maxrd, pt["rlo"][:], "is_ge")
            rr2 = w("rr2"); tt(rr2, maxrd, pt["rhi"][:], "is_le")
            inrr = w("inrr"); tt(inrr, rr1, rr2, "logical_and")
            aox = w("aox"); act(aox, ox, "Abs")
            aoy = w("aoy"); act(aoy, oy, "Abs")
            ga = w("ga"); tt(ga, aox, s15, "is_lt")
            gb = w("gb"); tt(gb, aoy, s15, "is_lt")
            ins0 = w("ins0"); ts(ins0, min4, 0.0, "is_gt")
            va = w("va"); tt(va, ins0, ga, "logical_and")
            vb = w("vb"); tt(vb, va, gb, "logical_and")
            valid = lw("valid"); tt(valid, vb, inrr, "logical_and")
            ox2 = w("ox2"); V.tensor_scalar(ox2, ox, w2r2, None, op0=AO.mult)
            oy2 = w("oy2"); V.tensor_scalar(oy2, oy, h2r2, None, op0=AO.mult)
            sq1 = w("sq1"); act(sq1, ox2, "Square")
            sq2 = w("sq2"); act(sq2, oy2, "Square")
            zz = w("zz"); tt(zz, sq1, sq2, "add")
            sroot = w("sroot"); act(sroot, zz, "Sqrt", bias=5e-9, scale=0.5)
            cent = lw("cent"); act(cent, sroot, "Relu", bias=1.0, scale=-1.0)

            # ================= prob term (matmul over classes) =================
            # probs loaded as [128, FR]: partition (grp, cls padded to 32)
            p128 = pool.tile([128, FR], dt, tag="p128")
            V.memset(p128[:], 0.0)
            for grp in range(4):
                nc.sync.dma_start(p128[grp * 32:grp * 32 + NCLS, :],
                                  probsT[:, grp * FR:(grp + 1) * FR])
            e128 = pool.tile([128, FR], dt, tag="e128")
            act(e128[:], p128[:], "Exp")
            labr = pool.tile([1, G], dt, tag="labr")
            nc.sync.dma_start(labr[:], glabrow[:])
            labb = pool.tile([NCLS, G], dt, tag="labb")
            GP.partition_broadcast(labb[:], labr[:], channels=NCLS)
            clsio = pool.tile([NCLS, 1], mybir.dt.int32, tag="clsio")
            GP.iota(clsio[:], pattern=[[1, 1]], base=0, channel_multiplier=1)
            clsf = pool.tile([NCLS, 1], dt, tag="clsf")
            V.tensor_copy(clsf[:], clsio[:])
            onehotT = pool.tile([NCLS, G], dt, tag="onehotT")
            V.tensor_scalar(onehotT[:], labb[:], clsf[:], None, op0=AO.is_equal)
            # block-diagonal lhsT [128, 128] (32-padded class blocks)
            oh60 = pool.tile([128, 128], dt, tag="oh60")
            V.memset(oh60[:], 0.0)
            on60 = pool.tile([128, 128], dt, tag="on60")
            V.memset(on60[:], 0.0)
            for grp in range(4):
                V.tensor_copy(oh60[grp * 32:grp * 32 + NCLS,
                                   grp * 32:(grp + 1) * 32], onehotT[:])
                V.memset(on60[grp * 32:grp * 32 + NCLS,
                              grp * 32:(grp + 1) * 32], 1.0)

            HF = FR // 2  # 341
            Et = lw("Et"); Zt = lw("Zt")
            for h in range(2):
                psE = psum.tile([128, HF], dt, tag="psE", bufs=2, name="psE%d" % h)
                psZ = psum.tile([128, HF], dt, tag="psZ", bufs=2, name="psZ%d" % h)
                rs = e128[:, h * HF:(h + 1) * HF]
                T.matmul(psE[:], oh60[:], rs, start=True, stop=True)
                T.matmul(psZ[:], on60[:], rs, start=True, stop=True)
                S.copy(Et[:, h * HF:(h + 1) * HF], psE[:])
                S.copy(Zt[:, h * HF:(h + 1) * HF], psZ[:])
            rz = w("rz"); V.reciprocal(rz, Zt)
            probterm = lw("probterm")
            stt(probterm, Et, 0.6, rz, "mult", "mult")

            # ================= IoU =================
            u_ = lw("u_")
            V.tensor_scalar(u_, cx1, c2, qoffx, op0=AO.mult, op1=AO.add)
            stt(u_, cy1, s2, u_, "mult", "add")
            v_ = lw("v_")
            V.tensor_scalar(v_, cx1, s2n, qoffy, op0=AO.mult, op1=AO.add)
            stt(v_, cy1, c2, v_, "mult", "add")
            cp = lw("cp")
            V.tensor_scalar(cp, c1, c2, None, op0=AO.mult)
            stt(cp, s1, s2, cp, "mult", "add")
            sp = lw("sp")
            V.tensor_scalar(sp, c1, s2, None, op0=AO.mult)
            stt(sp, s1, c2, sp, "mult", "subtract")
            acp = w("acp"); act(acp, cp, "Abs")
            asp = w("asp"); act(asp, sp, "Abs")
            acpc = lw("acpc"); ts(acpc, acp, 1e-12, "max")
            aspc = lw("aspc"); ts(aspc, asp, 1e-12, "max")
            sgc = lw("sgc"); ts(sgc, cp, 0.0, "is_ge", 2.0, "mult")
            ts(sgc, sgc, 1.0, "subtract")
            sgs = lw("sgs"); ts(sgs, sp, 0.0, "is_ge", 2.0, "mult")
            ts(sgs, sgs, 1.0, "subtract")
            nsgc = lw("nsgc"); ts(nsgc, sgc, -1.0, "mult")
            nsgs = lw("nsgs"); ts(nsgs, sgs, -1.0, "mult")
            pp = lw("pp"); tt(pp, acpc, aspc, "mult")
            rpp = lw("rpp")
            rpp_scr = w("rpp_scr")
            V.reciprocal_approx_accurate(rpp, pp, rpp_scr)

            CK1 = lw("CK1"); CK2 = lw("CK2")
            k1a = w("k1a"); tt(k1a, u_, sp, "mult")
            k1b = w("k1b"); tt(k1b, v_, cp, "mult")
            tt(CK1, k1a, k1b, "subtract")
            k2a = w("k2a"); tt(k2a, u_, cp, "mult")
            k2b = w("k2b"); tt(k2b, v_, sp, "mult")
            tt(CK2, k2a, k2b, "add")

            Stot = lw("Stot")
            first_con = [True]

            def accum(conv):
                if first_con[0]:
                    V.tensor_copy(Stot, conv)
                    first_con[0] = False
                else:
                    tt(Stot, Stot, conv, "add")

            def clip_pass_det():
                # subject: det box (Ag,Bg) axes (cp,sp); clip vs [-W,W]x[-H,H]
                Acp = w("Acp"); tt(Acp, Ag, cp, "mult")
                Asp = w("Asp"); tt(Asp, Ag, sp, "mult")
                Bcp = w("Bcp"); tt(Bcp, Bg, cp, "mult")
                Bsp = w("Bsp"); tt(Bsp, Bg, sp, "mult")
                xp = w("xp"); tt(xp, u_, Acp, "add")
                xm = w("xm"); tt(xm, u_, Acp, "subtract")
                yp = w("yp"); tt(yp, v_, Asp, "add")
                ym = w("ym"); tt(ym, v_, Asp, "subtract")
                x0s, y0s = [], []
                for e, (bx, sx) in enumerate([(xp, "add"), (xp, "subtract"),
                                              (xm, "subtract"), (xm, "add")]):
                    x0 = w("x0_%d" % e); tt(x0, bx, Bsp, sx); x0s.append(x0)
                for e, (by, sy) in enumerate([(yp, "subtract"), (yp, "add"),
                                              (ym, "add"), (ym, "subtract")]):
                    y0 = w("y0_%d" % e); tt(y0, by, Bcp, sy); y0s.append(y0)
                # cE per edge
                cEs = []
                for e, (ck, base, sub) in enumerate([
                        (CK2, Ag, False), (CK1, Bg, True),
                        (CK2, Ag, True), (CK1, Bg, False)]):
                    cE = w("cE_%d" % e)
                    if sub:   # base - ck
                        tt(cE, base, ck, "subtract")
                    else:     # ck + base
                        tt(cE, ck, base, "add")
                    cEs.append(cE)
                # negL per family: -2B*pp, -2A*pp
                B2n = w("B2n"); ts(B2n, Bg, -2.0, "mult")
                A2n = w("A2n"); ts(A2n, Ag, -2.0, "mult")
                negL0 = w("negL0"); tt(negL0, pp, B2n, "mult")
                negL1 = w("negL1"); tt(negL1, pp, A2n, "mult")
                sgx = [nsgs, nsgc, sgs, sgc]
                sgy = [sgc, nsgs, nsgc, sgs]
                aux = [aspc, acpc, aspc, acpc]
                auy = [acpc, aspc, acpc, aspc]
                negL = [negL0, negL1, negL0, negL1]
                for e in range(4):
                    wx = w("wx"); tt(wx, x0s[e], sgx[e], "mult")
                    wy = w("wy"); tt(wy, y0s[e], sgy[e], "mult")
                    nhix = w("nhix"); stt(nhix, wx, Wsc, auy[e], "subtract", "mult")
                    nhiy = w("nhiy"); stt(nhiy, wy, Hsc, aux[e], "subtract", "mult")
                    loxn = w("loxn"); stt(loxn, wx, Wsc, auy[e], "add", "mult")
                    loyn = w("loyn"); stt(loyn, wy, Hsc, aux[e], "add", "mult")
                    nt1a = w("nt1a"); tt(nt1a, nhix, nhiy, "max")
                    nt1 = w("nt1"); tt(nt1, nt1a, negL[e], "max")
                    mna = w("mna"); tt(mna, loxn, loyn, "min")
                    mn0 = w("mn0"); ts(mn0, mna, 0.0, "min")
                    dtr = w("dtr"); tt(dtr, mn0, nt1, "subtract")
                    dtp = w("dtp"); act(dtp, dtr, "Relu")
                    con = w("con"); tt(con, dtp, cEs[e], "mult")
                    accum(con)

            def clip_pass_gt():
                # subject: gt box (W,H) axes (cp,-sp) at (up,vp)=(-CK2,CK1);
                # clip vs [-Ag,Ag]x[-Bg,Bg]; corrections folded into cE.
                up = w("up"); ts(up, CK2, -1.0, "mult")
                vp = CK1
                # CKg1 = up*cp - vp*sp ; CKg2 = up*sp + vp*cp
                CKg1 = w("CKg1"); CKg2 = w("CKg2")
                g1a = w("g1a"); tt(g1a, up, cp, "mult")
                g1b = w("g1b"); tt(g1b, vp, sp, "mult")
                tt(CKg1, g1a, g1b, "subtract")
                g2a = w("g2a"); tt(g2a, up, sp, "mult")
                g2b = w("g2b"); tt(g2b, vp, cp, "mult")
                tt(CKg2, g2a, g2b, "add")
                # corners with sin=-sp: Asp_g = W*(-sp) etc (W,H scalars)
                Wcp = w("Wcp"); V.tensor_scalar(Wcp, cp, Wsc, None, op0=AO.mult)
                Wspn = w("Wspn"); V.tensor_scalar(Wspn, sp, Wsc, -1.0, op0=AO.mult, op1=AO.mult)
                Hcp = w("Hcp"); V.tensor_scalar(Hcp, cp, Hsc, None, op0=AO.mult)
                Hspn = w("Hspn"); V.tensor_scalar(Hspn, sp, Hsc, -1.0, op0=AO.mult, op1=AO.mult)
                xp = w("xpg"); tt(xp, up, Wcp, "add")
                xm = w("xmg"); tt(xm, up, Wcp, "subtract")
                yp = w("ypg"); tt(yp, vp, Wspn, "add")
                ym = w("ymg"); tt(ym, vp, Wspn, "subtract")
                x0s, y0s = [], []
                for e, (bx, sx) in enumerate([(xp, "add"), (xp, "subtract"),
                                              (xm, "subtract"), (xm, "add")]):
                    x0 = w("gx0_%d" % e); tt(x0, bx, Hspn, sx); x0s.append(x0)
                for e, (by, sy) in enumerate([(yp, "subtract"), (yp, "add"),
                                              (ym, "add"), (ym, "subtract")]):
                    y0 = w("gy0_%d" % e); tt(y0, by, Hcp, sy); y0s.append(y0)
                # cE with corrections: [CKg1+W+u, CKg2+H+v, -(CKg1-W+u), -(CKg2-H+v)]
                cEs = []
                for e, (ck, ssc, uv, neg) in enumerate([
                        (CKg1, Wsc, u_, False), (CKg2, Hsc, v_, False),
                        (CKg1, Wsc, u_, True), (CKg2, Hsc, v_, True)]):
                    cE = w("gcE_%d" % e)
                    if neg:
                        stt(cE, ck, ssc, uv, "subtract", "add")   # ck-W+u (negated later)
                    else:
                        stt(cE, ck, ssc, uv, "add", "add")        # ck+W+u
                    cEs.append(cE)
                # negL: -2H*pp, -2W*pp (scalars -h2, -w2 times pp)
                h2n = w("h2n"); V.tensor_scalar(h2n, pp, h2, -1.0, op0=AO.mult, op1=AO.mult)
                w2n = w("w2n"); V.tensor_scalar(w2n, pp, w2, -1.0, op0=AO.mult, op1=AO.mult)
                sgx = [sgs, nsgc, nsgs, sgc]
                sgy = [sgc, sgs, nsgc, nsgs]
                aux = [aspc, acpc, aspc, acpc]
                auy = [acpc, aspc, acpc, aspc]
                negL = [h2n, w2n, h2n, w2n]
                for e in range(4):
                    wx = w("gwx"); tt(wx, x0s[e], sgx[e], "mult")
                    wy = w("gwy"); tt(wy, y0s[e], sgy[e], "mult")
                    nhix = w("gnhix"); tt(nhix, wx, Ag, "subtract"); tt(nhix, nhix, auy[e], "mult")
                    nhiy = w("gnhiy"); tt(nhiy, wy, Bg, "subtract"); tt(nhiy, nhiy, aux[e], "mult")
                    loxn = w("gloxn"); tt(loxn, wx, Ag, "add"); tt(loxn, loxn, auy[e], "mult")
                    loyn = w("gloyn"); tt(loyn, wy, Bg, "add"); tt(loyn, loyn, aux[e], "mult")
                    nt1a = w("gnt1a"); tt(nt1a, nhix, nhiy, "max")
                    nt1 = w("gnt1"); tt(nt1, nt1a, negL[e], "max")
                    mna = w("gmna"); tt(mna, loxn, loyn, "min")
                    mn0 = w("gmn0"); ts(mn0, mna, 0.0, "min")
                    dtr = w("gdtr"); tt(dtr, mn0, nt1, "subtract")
                    dtp = w("gdtp"); act(dtp, dtr, "Relu")
                    con = w("gcon"); tt(con, dtp, cEs[e], "mult")
                    if e >= 2:
                        tt(Stot, Stot, con, "subtract")
                    else:
                        accum(con)

            clip_pass_det()
            clip_pass_gt()

            inter = w("inter")
            ts(inter, Stot, 0.5, "mult")
            tt(inter, inter, rpp, "mult")
            un = w("un")
            V.tensor_scalar(un, area1, whp, None, op0=AO.add)
            tt(un, un, inter, "subtract")
            ts(un, un, 1e-8, "max")
            run_ = w("run")
            run_scr = w("run_scr")
            V.reciprocal_approx_accurate(run_, un, run_scr)
            iou = lw("iou"); tt(iou, inter, run_, "mult")

            # ================= cost & per-core topk =================
            cost = lw("cost")
            stt(cost, cent, 0.2, probterm, "mult", "add")
            stt(cost, iou, 0.2, cost, "mult", "add")
            tt(cost, cost, valid, "mult")

            mx1 = sm("mx1", 8); mi1 = pool.tile([128, 8], mybir.dt.uint32, tag="mi1")
            V.max_with_indices(mx1[:, 0:8], mi1[:], cost)
            costc = w("costc")
            V.match_replace(costc, mx1[:, 0:8], cost, -1e30)
            mx2 = sm("mx2", 8); mi2 = pool.tile([128, 8], mybir.dt.uint32, tag="mi2")
            V.max_with_indices(mx2[:, 0:8], mi2[:], costc)
            gi1 = sm("gi1", 8); V.tensor_copy(gi1[:], mi1[:])
            V.tensor_scalar(gi1[:], gi1[:], basesc, None, op0=AO.add)
            gi2 = sm("gi2", 8); V.tensor_copy(gi2[:], mi2[:])
            V.tensor_scalar(gi2[:], gi2[:], basesc, None, op0=AO.add)
            cmx = sm("cmx", 8); cmi = pool.tile([128, 8], mybir.dt.uint32, tag="cmi")
            V.max_with_indices(cmx[:, 0:8], cmi[:], cent)
            cgi = sm("cgi", 8); V.tensor_copy(cgi[:], cmi[:])
            V.tensor_scalar(cgi[:], cgi[:], basesc, None, op0=AO.add)

            pay = pool.tile([128, 40], dt, tag="pay")
            V.tensor_copy(pay[:, 0:8], mx1[:, 0:8])
            V.tensor_copy(pay[:, 8:16], mx2[:, 0:8])
            V.tensor_copy(pay[:, 16:24], gi1[:])
            V.tensor_copy(pay[:, 24:32], gi2[:])
            V.tensor_copy(pay[:, 32:33], cmx[:, 0:1])
            V.tensor_copy(pay[:, 33:34], cgi[:, 0:1])
            V.memset(pay[:, 34:40], 0.0)
            GP.dma_start(g1in[:], pay[:])
            GP.collective_compute(
                "AllGather", mybir.AluOpType.bypass,
                replica_groups=[list(range(NCORES))],
                ins=[g1in[:]], outs=[g1out[:]])

            # reload: per partition (grp,g) -> g's candidates from all (core,grp')
            g1v = g1out[:].rearrange("(c p) k -> c p k", c=NCORES)  # [8,128,40]
            # vals [128, 8*4*16]; same [32, 512] content loaded to each grp block
            vals = pool.tile([128, 512], dt, tag="vals")
            src = g1v[:, :, 0:16].rearrange("c (a b) k -> b c a k", a=4)  # [32,8,4,16]
            for x in range(4):
                nc.sync.dma_start(
                    vals[x * 32:(x + 1) * 32, :].rearrange(
                        "p (c a k) -> p c a k", c=NCORES, a=4), src)
            vmx1 = sm("vmx1", 8); V.max(vmx1[:, 0:8], vals[:])
            valsc = pool.tile([128, 512], dt, tag="valsc")
            V.match_replace(valsc[:], vmx1[:, 0:8], vals[:], -1e30)
            vmx2 = sm("vmx2", 8); V.max(vmx2[:, 0:8], valsc[:])
            thr = sm("thr"); V.tensor_copy(thr[:], vmx2[:, 6:7])  # 15th largest
            threff = sm("threff"); ts(threff[:], thr[:], 1e-20, "max")

            # cent global argmax: vals at col 32, idx at col 33
            cvals = pool.tile([128, 32], dt, tag="cvals")
            srcv = g1v[:, :, 32:33].rearrange("c (a b) k -> b (c a k)", a=4)
            cidxs = pool.tile([128, 32], dt, tag="cidxs")
            srci = g1v[:, :, 33:34].rearrange("c (a b) k -> b (c a k)", a=4)
            for x in range(4):
                nc.sync.dma_start(cvals[x * 32:(x + 1) * 32, :], srcv)
                nc.sync.dma_start(cidxs[x * 32:(x + 1) * 32, :], srci)
            cm_ = sm("cm_")
            V.tensor_reduce(cm_[:], cvals[:], axis=mybir.AxisListType.X, op=AO.max)
            ceq = pool.tile([128, 32], dt, tag="ceq")
            V.tensor_scalar(ceq[:], cvals[:], cm_[:], None, op0=AO.is_equal)
            cbig = pool.tile([128, 32], dt, tag="cbig")
            ts(cbig[:], ceq[:], -1e9, "mult", 1e9, "add")  # (1-eq)*1e9
            cnd = pool.tile([128, 32], dt, tag="cnd")
            tt(cnd[:], ceq[:], cidxs[:], "mult")
            tt(cnd[:], cnd[:], cbig[:], "add")
            anch = sm("anch")
            V.tensor_reduce(anch[:], cnd[:], axis=mybir.AxisListType.X, op=AO.min)
            anchloc = sm("anchloc")
            V.tensor_scalar(anchloc[:], anch[:], n0sc, None, op0=AO.subtract)

            # ================= selection tail =================
            sel = w("sel")
            V.tensor_scalar(sel, cost, threff[:], None, op0=AO.is_ge)
            areas = lw("areas")
            V.tensor_scalar(areas, sel, whp, None, op0=AO.mult)
            amax = lw("amax")
            blockmax(amax, areas)
            eq = w("eq"); tt(eq, areas, amax, "is_equal")
            gq = lw("gq")
            V.tensor_scalar(gq, eq, s32mg, 1.0, op0=AO.mult, op1=AO.subtract)
            gqm = lw("gqm")
            blockmax(gqm, gq)
            oh = lw("oh"); tt(oh, gq, gqm, "is_equal")
            matchedT = lw("matchedT"); ts(matchedT, amax, 0.0, "is_gt")
            ohm = lw("ohm"); tt(ohm, oh, matchedT, "mult")
            cntp = sm("cntp")
            V.tensor_reduce(cntp[:], ohm, axis=mybir.AxisListType.X, op=AO.add)
            pay2 = pool.tile([128, 2], dt, tag="pay2")
            V.tensor_copy(pay2[:, 0:1], cntp[:])
            V.memset(pay2[:, 1:2], 0.0)
            GP.dma_start(g2in[:], pay2[:])
            GP.collective_compute(
                "AllGather", mybir.AluOpType.bypass,
                replica_groups=[list(range(NCORES))],
                ins=[g2in[:]], outs=[g2out[:]])
            g2v = g2out[:].rearrange("(c p) k -> c p k", c=NCORES)
            cnt_all = pool.tile([128, 32], dt, tag="cnt_all")
            srcc = g2v[:, :, 0:1].rearrange("c (a b) k -> b (c a k)", a=4)
            for x in range(4):
                nc.sync.dma_start(cnt_all[x * 32:(x + 1) * 32, :], srcc)
            cnts = sm("cnts")
            V.tensor_reduce(cnts[:], cnt_all[:], axis=mybir.AxisListType.X, op=AO.add)
            cnt0 = sm("cnt0"); ts(cnt0[:], cnts[:], 0.0, "is_equal")

            # ================= rematch + final one-hot =================
            fix = w("fix")
            V.tensor_scalar(fix, idxloc, anchloc[:], cnt0[:],
                            op0=AO.is_equal, op1=AO.logical_and)
            fixg = w("fixg")
            V.tensor_scalar(fixg, fix, gp1sc, None, op0=AO.mult)
            fgm = lw("fgm")
            blockmax(fgm, fixg)
            anyfixT = w("anyfixT"); ts(anyfixT, fgm, 0.0, "is_gt")
            fixu = w("fixu"); tt(fixu, fixg, fgm, "is_equal")
            fixf = w("fixf"); tt(fixf, fixu, anyfixT, "mult")
            nanf = w("nanf"); ts(nanf, anyfixT, -1.0, "mult", 1.0, "add")
            ohm2 = w("ohm2"); tt(ohm2, ohm, nanf, "mult")
            ohf = lw("ohf"); tt(ohf, ohm2, fixf, "add")
            # default g0 one-hot for unassigned points
            notm = w("notm"); ts(notm, matchedT, -1.0, "mult", 1.0, "add")
            dm = lw("dm"); tt(dm, notm, nanf, "mult")
            dmg0 = w("dmg0")
            V.tensor_scalar(dmg0, dm, isg0, None, op0=AO.mult)
            ohd = lw("ohd"); tt(ohd, ohf, dmg0, "add")

            # ================= gathers (matmul over g) + outputs =================
            HF2 = HF  # 341
            ohd_rs = lw("ohd_rs"); tt(ohd_rs, ohd, rsS, "mult")  # pre-scaled by 1/stride
            prods = []
            for nm, srcT in [("l", l_), ("t", tb), ("r", r_), ("b", b_)]:
                p_ = lw("p_" + nm); tt(p_, ohd_rs, srcT, "mult")
                prods.append((nm, p_))
            grpOH = gs[:, 12:16]                 # [128, 4] grp one-hot selector
            lab4 = pool.tile([128, 4], dt, tag="lab4")
            V.tensor_scalar(lab4[:], grpOH, labv, None, op0=AO.mult)
            at4 = pool.tile([128, 4], dt, tag="at4")
            V.tensor_scalar(at4[:], grpOH, a2r, None, op0=AO.mult)
            lab15 = sm("lab15")
            ts(lab15[:], labv, -1.0, "mult", 15.0, "add")     # 15 - lab_g
            lab4f = pool.tile([128, 4], dt, tag="lab4f")
            V.tensor_scalar(lab4f[:], grpOH, lab15[:], None, op0=AO.mult)

            # quantities to gather: (name, lhsT [128,4], rhs wide tile)
            quants = [("lab", lab4[:], ohd), ("labf", lab4f[:], dmg0),
                      ("at", at4[:], ohd)]
            for nm, p_ in prods:
                quants.append(("q" + nm, grpOH, p_))

            gout = {}
            for qi, (nm, lhsT, rhsT) in enumerate(quants):
                res = pool.tile([4, FR], dt, tag="g_" + nm, name="g_" + nm)
                for h in range(2):
                    ps_ = psum.tile([4, HF2], dt, tag="gps", bufs=4,
                                    name="gps_%s_%d" % (nm, h))
                    T.matmul(ps_[:], lhsT,
                             rhsT[:, h * HF2:(h + 1) * HF2],
                             start=True, stop=True)
                    S.copy(res[:, h * HF2:(h + 1) * HF2], ps_[:])
                gout[nm] = res

            # labels = gather(ohf*lab) + gather(dmg0*(15-lab))
            labf = pool.tile([4, FR], dt, tag="labf")
            tt(labf[:], gout["lab"][:], gout["labf"][:], "add")
            labi = pool.tile([4, FR], mybir.dt.int32, tag="labi")
            V.tensor_copy(labi[:], labf[:])
            nc.sync.dma_start(out_labels[:].rearrange("(a f) -> a f", a=4), labi[:])

            for k, nm in enumerate(["ql", "qt", "qr", "qb"]):
                dstv = out_bt[:, k:k + 1].rearrange("(a f) o -> a (f o)", a=4)
                nc.sync.dma_start(dstv, gout[nm][:])
            dsta = out_at[:, 0:1].rearrange("(a f) o -> a (f o)", a=4)
            nc.sync.dma_start(dsta, gout["at"][:])

    nc.compile()
    return nc, dbg


def _prep_inputs(inputs):
    """Shard + replicate host-side (pure layout work)."""
    f32 = np.float32
    points = np.asarray(inputs["points"], f32)
    rr = np.asarray(inputs["regress_ranges"], f32)
    spp = np.asarray(inputs["stride_pp"], f32)
    gtb = np.asarray(inputs["gt_bboxes"], f32)
    gtl = np.asarray(inputs["gt_labels"])
    bp = np.asarray(inputs["bbox_preds"], f32)
    probs = np.asarray(inputs["probs"], f32)

    fields = [points[:, 0], points[:, 1], bp[:, 0], bp[:, 1], bp[:, 2],
              bp[:, 3], bp[:, 4], spp, rr[:, 0], rr[:, 1]]
    probsT_full = np.ascontiguousarray(probs.T)  # [15, N]

    g_idx = np.arange(128) % 32
    grp_idx = np.arange(128) // 32
    gsc_base = np.zeros((128, 16), f32)
    gsc_base[:, 0] = gtb[g_idx, 0]
    gsc_base[:, 1] = gtb[g_idx, 1]
    gsc_base[:, 2] = gtb[g_idx, 2]
    gsc_base[:, 3] = gtb[g_idx, 3]
    gsc_base[:, 4] = gtb[g_idx, 4]
    gsc_base[:, 5] = gtl[g_idx].astype(f32)
    gsc_base[:, 6] = (grp_idx * FR).astype(f32)
    gsc_base[:, 8] = (32 - g_idx).astype(f32)
    gsc_base[:, 9] = (g_idx + 1).astype(f32)
    gsc_base[:, 10] = (g_idx == 0).astype(f32)
    for grp in range(4):
        gsc_base[:, 12 + grp] = (grp_idx == grp).astype(f32)
    glabrow = gtl.astype(f32).reshape(1, G)

    in_maps = []
    for c in range(NCORES):
        sl = slice(c * SH, (c + 1) * SH)
        ptf = np.empty((10, SH), f32)
        for i, arr in enumerate(fields):
            ptf[i] = arr[sl]
        gsc = gsc_base.copy()
        gsc[:, 7] = c * SH
        in_maps.append({
            "ptf": ptf,
            "probsT": np.ascontiguousarray(probsT_full[:, sl]),
            "gsc": gsc,
            "glabrow": glabrow,
        })
    return in_maps


def _get_runner():
    """Build + compile once; return a cached callable in_maps -> per-core outs."""
    if "runner" in _cache:
        return _cache["runner"]
    import jax
    import concourse.mybir as mybir
    from jax.experimental.shard_map import shard_map
    from jax.sharding import Mesh, PartitionSpec
    from concourse.bass2jax import (_bass_exec_p, install_neuronx_cc_hook,
                                    partition_id_tensor)

    if "nc" not in _cache:
        _cache["nc"], _ = _build()
    nc = _cache["nc"]
    install_neuronx_cc_hook()

    in_names, out_names, out_avals, zero_outs = [], [], [], []
    partition_name = nc.partition_id_tensor.name if nc.partition_id_tensor else None
    for alloc in nc.m.functions[0].allocations:
        if not isinstance(alloc, mybir.MemoryLocationSet):
            continue
        name = alloc.memorylocations[0].name
        if alloc.kind == "ExternalInput":
            if name != partition_name:
                in_names.append(name)
        elif alloc.kind == "ExternalOutput":
            shape = tuple(alloc.tensor_shape)
            dtype = mybir.dt.np(alloc.dtype)
            out_names.append(name)
            out_avals.append(jax.core.ShapedArray(shape, dtype))
            zero_outs.append(np.zeros((NCORES * shape[0],) + shape[1:], dtype))
    n_params = len(in_names)
    dbg_name = nc.dbg_addr.name if nc.dbg_addr is not None else None
    full_names = tuple(in_names + out_names
                       + ([partition_name] if partition_name else []))

    def _body(*args):
        operands = list(args)
        if partition_name is not None:
            operands.append(partition_id_tensor())
        outs = _bass_exec_p.bind(
            *operands,
            out_avals=tuple(out_avals),
            in_names=full_names,
            out_names=tuple(out_names),
            lowering_input_output_aliases=(),
            sim_require_finite=True,
            sim_require_nnan=True,
            nc=nc,
        )
        return tuple(outs)

    devices = jax.devices()[:NCORES]
    mesh = Mesh(np.asarray(devices), ("core",))
    donate = tuple(range(n_params, n_params + len(out_names)))
    sharded = jax.jit(
        shard_map(_body, mesh=mesh,
                  in_specs=(PartitionSpec("core"),) * (n_params + len(out_names)),
                  out_specs=(PartitionSpec("core"),) * len(out_names),
                  check_rep=False),
        donate_argnums=donate, keep_unused=True)

    dbg_zero = np.zeros((1, 2), np.uint32)

    def run(in_maps):
        concat_in = [
            np.concatenate([np.asarray(m.get(name, dbg_zero)) for m in in_maps],
                           axis=0)
            for name in in_names]
        zouts = [z.copy() for z in zero_outs]
        out_arrs = sharded(*concat_in, *zouts)
        outs = {}
        for i, name in enumerate(out_names):
            a = np.asarray(out_arrs[i])
            outs[name] = a.reshape((NCORES,) + tuple(out_avals[i].shape))
        return outs

    _cache["runner"] = run
    return run


def kernel(**inputs):
    run = _get_runner()
    in_maps = _prep_inputs(inputs)
    outs = run(in_maps)
    labels = outs["out_labels"].reshape(N)
    bt = outs["out_bt"].reshape(N, 4)
    at = outs["out_at"].reshape(N, 1)
    return labels.astype(np.int32), bt, at
